# revision 16
# baseline (speedup 1.0000x reference)
"""Trainium2 Bass kernel for nn_DeTokenizer (EMA detokenizer).

Computation (forward):
    p_s      = clip(router_probs[0, tok_idx, 1], EPS, 1-EPS)         (M,)
    h_m      = (1-p_m) h_{m-1} + p_m * hidden[m]     (EMA over M chunks, D channels)
    out[t]   = residual[t] + coef[t] * h[j(t)]       j(t) = cumsum(mask)-1
    coef[t]  = mx + (1 - mx)  (straight-through; == 1 up to 2^-24 in f32 forward,
               which is far below the fp16 noise floor, so it is omitted)

Strategy: the EMA is linear, so h_m = sum_s exp(LC_m - LC_s) * p_s * hidden[s]
with LC = cumsum(log(1-p)) computed on host in f64 from the (scalar) router
metadata. Each of the 8 cores owns M/8 chunks, processed as blocks of 128:
a [128,128] triangular band matrix (host-built constant) matmul against the
block's hidden tile, plus NW window matmuls against the preceding 128-chunk
tiles (contributions older than NW*128 chunks decay like exp(sum log a) —
verified on host against the actual data, NW escalated if needed). No
collectives and no serial carry chain: cross-core dependence is covered by a
halo of NW*128 hidden rows.

The problem is HBM-bandwidth bound (res in + out out dominate), so all
device-side I/O is fp16 (tolerance is 2e-2 rel; fp16 end-to-end costs ~3e-4).
Queue/engine choreography per the measured traces:
  - SDMA engines round-robin service across ACTIVE queues at packet
    granularity, so bytes-per-queue balance decides who starves. Loads
    (hid+mats image, res) are split across BOTH HWDGE queues (scalar+sync),
    emitted up front so queue FIFO order = priority. Stores go to the SWDGE
    (gpsimd) queue, except the last two blocks' stores, which ride behind the
    loads on the HWDGE queues (loads are done by then, giving a 3-queue
    store drain at the end).
  - res/out tiles are per-block [128, R*D]: a partition row is R=4
    consecutive tokens = 8KB contiguous DRAM (1MB loads); stores are per-r
    [128, D] slices so the tail after the last res arrival is one DVE add
    plus a 256KB store.
  - the EMA result is cast PSUM->SBUF fp16 once per block on the DVE, and
    the residual adds run all-SBUF fp16 in 2x perf mode (a PSUM operand
    would cap tensor_tensor at 1x with a +120cyc penalty).
"""

import numpy as np

EPS = 1e-4
N_CORES = 8
P = 128  # SBUF partitions / block size
NMAX = 512  # max matmul free dim (one PSUM bank of f32)
DECAY_TOL = 1e-10

_NC_CACHE: dict = {}


def _build(NB: int, NW: int, D: int, R: int, Lc: int):
    """Build + compile the per-core Bass program (same NEFF for all cores)."""
    import concourse.bacc as bacc
    import concourse.mybir as mybir
    import concourse.tile as tile

    f16 = mybir.dt.float16
    f32 = mybir.dt.float32
    add = mybir.AluOpType.add

    nc = bacc.Bacc("TRN2", target_bir_lowering=False, debug=False,
                   num_devices=N_CORES)
    NH = NB + NW               # hid 128-row tiles (incl. halo)
    S = NW + 1                 # mat sub-blocks per block
    NHA = NW + 1               # hid tiles in image A (just enough for b=0)
    WA = NB * S * P + NHA * D  # image A: [mats | hid tiles 0..NHA-1]
    WB = (NH - NHA) * D        # image B: [hid tiles NHA..NH-1]
    hma = nc.dram_tensor("hma", [P, WA], f16, kind="ExternalInput").ap()
    hmb = nc.dram_tensor("hmb", [P, WB], f16, kind="ExternalInput").ap()
    res = nc.dram_tensor("res", [Lc, D], f16, kind="ExternalInput").ap()
    out = nc.dram_tensor("out", [Lc, D], f16, kind="ExternalOutput").ap()

    nsplit = (D + NMAX - 1) // NMAX
    HOFF = NB * S * P          # hid offset inside image A

    with tile.TileContext(nc) as tc:
        with tc.tile_pool(name="hmap", bufs=1) as hmap, \
             tc.tile_pool(name="hmbp", bufs=1) as hmbp, \
             tc.tile_pool(name="psum", bufs=4, space="PSUM") as ppool, \
             tc.tile_pool(name="hp", bufs=4) as hpool, \
             tc.tile_pool(name="resp", bufs=NB) as rpool:
            hma_t = hmap.tile([P, WA], f16)
            hmb_t = hmbp.tile([P, WB], f16)
            # hma (mats + the two hid tiles block 0 needs) leads the scalar
            # ring; hmb rides the sync ring AFTER res1 so block 0's operands
            # complete at minimum aggregate-bandwidth time.
            nc.scalar.dma_start(out=hma_t[:], in_=hma)

            # per-block load/store views (8KB contiguous per partition row)
            # and per-(block, r) store views (2KB runs) for the tail blocks
            resv = res.rearrange("(b p r) d -> b p (r d)", p=P, r=R)
            outv = out.rearrange("(b p r) d -> b p (r d)", p=P, r=R)
            outvr = out.rearrange("(b p r) d -> b r p d", p=P, r=R)

            # Load order per ring (FIFO = data arrival order): scalar
            # [hma, res0, res2, res4, res6], sync [res1, hmb, res3, res5,
            # res7] — byte-balanced rings, res0/res1 land right after the
            # hm images. With 8 DMA semaphore lanes and 10 loads, the two
            # lane-reuse waits are pushed onto res5/res7 (emitted last),
            # where they are harmless; the ACT copies queued behind the
            # scalar loads issue unblocked.
            rts = [rpool.tile([P, R * D], f16, tag="res", name=f"rt{b}")
                   for b in range(NB)]
            nc.scalar.dma_start(out=rts[0][:], in_=resv[0])
            nc.sync.dma_start(out=rts[1][:], in_=resv[1])
            nc.sync.dma_start(out=hmb_t[:], in_=hmb)
            for b in (2, 4, 6):
                nc.scalar.dma_start(out=rts[b][:], in_=resv[b])
            for b in (3, 5, 7):
                nc.sync.dma_start(out=rts[b][:], in_=resv[b])

            def hid_slice(i, c0, c1):
                if i < NHA:
                    return hma_t[:, HOFF + i * D + c0:HOFF + i * D + c1]
                return hmb_t[:, (i - NHA) * D + c0:(i - NHA) * D + c1]

            for b in range(NB):
                ps = ppool.tile([P, D], f32, tag="ps")
                for n in range(nsplit):
                    c0, c1 = n * NMAX, min((n + 1) * NMAX, D)
                    for w in range(S):
                        # w=0: diagonal (triangular) block on own tile;
                        # w>=1: window block on the w-th preceding tile.
                        nc.tensor.matmul(
                            ps[:, c0:c1],
                            lhsT=hma_t[:, (b * S + w) * P:(b * S + w + 1) * P],
                            rhs=hid_slice(b + NW - w, c0, c1),
                            start=(w == 0),
                            stop=(w == NW),
                        )
                h = hpool.tile([P, D], f16, tag="h")
                # PSUM->SBUF fp16 cast on ACT: keeps the DVE stream (which
                # paces the second half of the kernel) down to the adds.
                nc.scalar.copy(out=h[:], in_=ps[:])
                rt = rts[b]
                if b < NB - 4:
                    # SWDGE issue is ~1.6us/DMA (serial on Q7): one 1MB
                    # store per block keeps the gpsimd queue off the
                    # critical path.
                    for r in range(R):
                        sl = slice(r * D, (r + 1) * D)
                        nc.vector.tensor_tensor(
                            out=rt[:, sl], in0=rt[:, sl], in1=h[:], op=add)
                    nc.gpsimd.dma_start(out=outv[b], in_=rt[:])
                elif b < NB - 2:
                    # spread the late store drain across all three queues
                    seng = nc.scalar if b % 2 == 0 else nc.sync
                    for r in range(R):
                        sl = slice(r * D, (r + 1) * D)
                        nc.vector.tensor_tensor(
                            out=rt[:, sl], in0=rt[:, sl], in1=h[:], op=add)
                    seng.dma_start(out=outv[b], in_=rt[:])
                else:
                    # last-arriving blocks: per-r 256KB stores on the (now
                    # drained) HWDGE load rings minimize the tail.
                    seng = nc.scalar if b % 2 == 0 else nc.sync
                    for r in range(R):
                        sl = slice(r * D, (r + 1) * D)
                        nc.vector.tensor_tensor(
                            out=rt[:, sl], in0=rt[:, sl], in1=h[:], op=add)
                        seng.dma_start(out=outvr[b, r], in_=rt[:, sl])
    nc.compile()
    return nc


def kernel(hidden_states, residual, token_mask, router_probs):
    from concourse import bass_utils

    hidden_states = np.asarray(hidden_states)
    residual = np.asarray(residual)
    token_mask = np.asarray(token_mask)
    router_probs = np.asarray(router_probs)

    _, M, D = hidden_states.shape
    _, L, _ = residual.shape
    assert M % (N_CORES * P) == 0 and L % M == 0
    R = L // M
    Mc = M // N_CORES      # chunks per core
    Lc = L // N_CORES      # tokens per core
    NB = Mc // P           # 128-chunk blocks per core

    mask = token_mask[0]
    j_map = np.clip(np.cumsum(mask.astype(np.int64)) - 1, 0, M - 1)
    assert np.array_equal(j_map, np.arange(L) // R), \
        "kernel requires uniform chunk lengths (mask = arange(L) % R == 0)"

    # ---- host scalar metadata (f64) ----
    p32 = router_probs[0, :, 1].astype(np.float32)
    tok_idx = np.nonzero(mask)[0]
    cp32 = np.clip(p32[tok_idx], np.float32(EPS), np.float32(1.0 - EPS))
    cp = cp32.astype(np.float64)
    la = np.log1p(-cp)
    LCx = np.concatenate([[0.0], np.cumsum(la)])  # LCx[i+1] = LC_i ; LCx[0]=0

    maxhid = float(np.abs(hidden_states).max()) or 1.0

    # pick NW: contributions older than NW*P chunks must be < DECAY_TOL
    NW = 1
    while NW < 4:
        g0s = np.arange(NB * N_CORES) * P
        g0s = g0s[g0s - NW * P > 0]
        worst = np.max(np.exp(LCx[g0s] - LCx[g0s - NW * P])) if g0s.size else 0.0
        if worst * maxhid < DECAY_TOL:
            break
        NW += 1
    S = NW + 1
    NH = NB + NW
    NHA = NW + 1

    # ---- per-core constants ----
    def band_mats(k):
        m0 = np.zeros((NB * S, P, P), np.float64)
        for b in range(NB):
            g0 = k * Mc + b * P
            m_idx = np.arange(g0, g0 + P)
            for w in range(S):
                s_idx = m_idx - w * P
                valid = s_idx >= 0
                sc = np.where(valid, s_idx, 0)
                blk = (np.exp(LCx[m_idx + 1][None, :] - LCx[sc + 1][:, None])
                       * cp[sc][:, None])
                if w == 0:
                    blk = np.where(s_idx[:, None] <= m_idx[None, :], blk, 0.0)
                blk = np.where(valid[:, None], blk, 0.0)
                m0[b * S + w] = blk
        # partition-major SBUF image: [s, (b*S+w)*P + m]
        return np.ascontiguousarray(
            m0.transpose(1, 0, 2).reshape(P, NB * S * P)).astype(np.float16)

    hid0 = hidden_states[0]
    res16 = residual[0].astype(np.float16)
    in_maps = []
    for k in range(N_CORES):
        lo = k * Mc - NW * P
        if lo < 0:
            halo = np.concatenate(
                [np.zeros((-lo, D), np.float32), hid0[:max(0, k * Mc)]])
        else:
            halo = hid0[lo:k * Mc]
        hid_k = np.concatenate([halo, hid0[k * Mc:(k + 1) * Mc]], axis=0)
        # partition-major SBUF image: [p, i*D + d] = chunk i*P+p, feature d
        hid_pm = np.ascontiguousarray(
            hid_k.reshape(NH, P, D).transpose(1, 0, 2).reshape(P, NH * D)
        ).astype(np.float16)
        in_maps.append({
            "hma": np.concatenate([band_mats(k), hid_pm[:, :NHA * D]], axis=1),
            "hmb": np.ascontiguousarray(hid_pm[:, NHA * D:]),
            "res": np.ascontiguousarray(res16[k * Lc:(k + 1) * Lc]),
        })

    key = (NB, NW, D, R, Lc)
    if key not in _NC_CACHE:
        _NC_CACHE[key] = _build(*key)
    nc = _NC_CACHE[key]

    results = bass_utils.run_bass_kernel_spmd(
        nc, in_maps, core_ids=list(range(N_CORES)))

    out_full = np.empty((1, L, D), np.float32)
    for k in range(N_CORES):
        out_full[0, k * Lc:(k + 1) * Lc] = results.results[k]["out"]
    return out_full
